# revision 16
# baseline (speedup 1.0000x reference)
"""Single-head encoder attention block on 8 Trainium2 NeuronCores.

Math (per batch element b):
    q = x @ wq.T ; k = x @ wk.T ; v = x @ wv.T
    scores = (q @ k.T) / sqrt(1024) ; attn = softmax(scores, -1)
    out = (attn @ v) @ wo.T

Sharding: data-parallel over batch - batch 8 maps 1:1 onto the 8 cores;
weights replicated. No collectives.

Per-core algorithm (matmul operands fp16; PSUM accumulation fp32; host
prepares device inputs: fp16 casts plus xT / woT layout):
  Two weight-product folds remove all operand transposes on device:
      scores = x (wq.T wk) x.T / 32            M  := wq.T @ wk
      attn @ v @ wo.T = attn @ x @ (wo wv).T   via UT[d,do] = sum_vc wv[vc,d] woT[vc,do]
  fp16 operands run every matmul at 1 cycle/row, halve input DMA
  (12MB/core), and let F = M @ xT stay fully resident in SBUF (no DRAM
  spill).  The 1/sqrt(dk)=1/32 scale is folded into the Exp activation.
  Rowsums accumulate on the Vector engine (tensor_add chain) so PE pays a
  single 512-wide ones-matmul per superblock.
  Phase A (DMA rings: scalar=xT; sync=woT; gpsimd/vector=wv,wk,wq):
    A0: UT h0 = wv-nat x woT-h0; Z h0 (xT quarters stream in)
    A1: UT h1, Z h1
    A2: M = wq.T @ wk ; A3: F = M @ xT (resident, unscaled)
  Phase B (per i-superblock of SB=512):
    scoresT[j,i] = sum_d2 xT[d2,j]*F[d2,i]; expT = exp(scoresT/32)
    out[i,do] = (sum_j expT[j,i-tile] * Z[j,do]) * recip[i]  (expT stationary);
    out-group 0 runs before the rowsum matmul so PE rides through the
    last exp's latency.
"""

import os
import sys

for _p in ("/opt/trn_rl_repo", "/root/.axon_site/_ro/trn_rl_repo"):
    if os.path.isdir(_p) and _p not in sys.path:
        sys.path.insert(0, _p)

import numpy as np
from contextlib import ExitStack

import concourse.bacc as bacc
import concourse.tile as tile
from concourse import mybir, masks
from concourse.bass_utils import run_bass_kernel_spmd

P = 128
S = 2048          # sequence length (per core)
D = 1024          # model dim = dk = dv
NS = S // P       # 16 seq tiles
ND = D // P       # 8 dim tiles
SB = 512          # i-superblock width (query columns per block)
NSB = S // SB     # 4 superblocks
NIT = SB // P     # 4 i-tiles per superblock
SCALE = 1.0 / 32.0  # 1/sqrt(1024)
N_CORES = 8

DT = mybir.dt.float32
MM = mybir.dt.float16
R32 = mybir.dt.float32r
F32 = mybir.dt.float32
EXP = mybir.ActivationFunctionType.Exp
COPY = mybir.ActivationFunctionType.Copy


def _build():
    nc = bacc.Bacc("TRN2", target_bir_lowering=False, debug=False, num_devices=N_CORES)

    xt_in = nc.dram_tensor("xt", [D, S], MM, kind="ExternalInput").ap()
    wq_in = nc.dram_tensor("wq", [D, D], MM, kind="ExternalInput").ap()
    wk_in = nc.dram_tensor("wk", [D, D], MM, kind="ExternalInput").ap()
    wv_in = nc.dram_tensor("wv", [D, D], MM, kind="ExternalInput").ap()
    wot_in = nc.dram_tensor("wot", [D, D], MM, kind="ExternalInput").ap()
    out_d = nc.dram_tensor("out", [S, D], DT, kind="ExternalOutput").ap()

    mm = nc.tensor.matmul

    with tile.TileContext(nc) as tc, ExitStack() as top:
        cst = top.enter_context(tc.tile_pool(name="cst", bufs=1))
        ident_f32 = cst.tile([P, P], DT)
        masks.make_identity(nc, ident_f32[:])
        ones_f32 = cst.tile([P, 1], DT)
        nc.gpsimd.memset(ones_f32[:], 1.0)
        ones_r = cst.tile([P, 1], R32)
        nc.vector.tensor_copy(ones_r[:], ones_f32[:])

        res1 = top.enter_context(tc.tile_pool(name="res1", bufs=1))
        xt = res1.tile([P, ND * S], MM)    # xT: tile d -> [:, d*S:(d+1)*S] = [d-part, s]
        res2 = top.enter_context(tc.tile_pool(name="res2", bufs=1))
        zres = res2.tile([P, NS * D], MM)  # Z: tile j -> [:, j*D:(j+1)*D] = [j-part, do]
        res3 = top.enter_context(tc.tile_pool(name="res3", bufs=1))
        fres = res3.tile([P, ND * S], MM)  # F: tile d2 -> [:, d2*S:(d2+1)*S] = [d2-part, i]

        with ExitStack() as pall:
            wgt = pall.enter_context(tc.tile_pool(name="wgt", bufs=1))

            wvn = wgt.tile([P, ND * D], MM)    # wv natural: vc-tile t -> [:, t*D:(t+1)*D]
            wkn = wgt.tile([P, ND * D], MM)    # wk natural
            wqn = wgt.tile([P, ND * D], MM)    # wq natural: ct-tile t -> [:, t*D + d1]
            wot = wgt.tile([P, ND * D], MM)    # woT: vc-tile t -> [:, t*D + do]

            # ---- front-load all input DMAs ----
            # woT h0 first (UT h0 is PE's first work); wv split across rings
            for t in range(ND):
                nc.sync.dma_start(
                    out=wot[:, t * D: t * D + 512],
                    in_=wot_in[t * P:(t + 1) * P, 0:512])
            for t in range(0, ND, 2):
                nc.gpsimd.dma_start(out=wvn[:, t * D:(t + 1) * D], in_=wv_in[t * P:(t + 1) * P, :])
            for t in range(1, ND, 2):
                nc.sync.dma_start(out=wvn[:, t * D:(t + 1) * D], in_=wv_in[t * P:(t + 1) * P, :])
            for t in range(ND):
                nc.sync.dma_start(
                    out=wot[:, t * D + 512: t * D + D],
                    in_=wot_in[t * P:(t + 1) * P, 512:D])
            # xT by s-quarters so Z chains can start after the first quarter
            for sq in range(4):
                for d in range(ND):
                    nc.scalar.dma_start(
                        out=xt[:, d * S + sq * 512: d * S + (sq + 1) * 512],
                        in_=xt_in[d * P:(d + 1) * P, sq * 512:(sq + 1) * 512])
            for t in range(ND):
                nc.gpsimd.dma_start(out=wkn[:, t * D:(t + 1) * D], in_=wk_in[t * P:(t + 1) * P, :])
            for t in range(ND):
                nc.gpsimd.dma_start(out=wqn[:, t * D:(t + 1) * D], in_=wq_in[t * P:(t + 1) * P, :])

            # ---------------- Phase A0/A1: UT halves, Z halves ----------------
            with ExitStack() as pw:
                mmps = pw.enter_context(tc.tile_pool(name="mmps", bufs=8, space="PSUM"))
                hwork = pw.enter_context(tc.tile_pool(name="hwork", bufs=1))

                def ut_compute(h):
                    # vc-major accumulation across 8 PSUM banks: PE consumes
                    # each wv/woT tile as it lands instead of waiting for all
                    ut_h = hwork.tile([P, ND * 512], MM, name=f"uth{h}", tag="ut", bufs=2)
                    pss = [mmps.tile([P, 512], F32, name=f"utp{h}{d}", tag="mm")
                           for d in range(ND)]
                    for vc in range(ND):
                        for d in range(ND):
                            mm(pss[d][:],
                               wvn[:, vc * D + d * P: vc * D + (d + 1) * P],
                               wot[:, vc * D + h * 512: vc * D + (h + 1) * 512],
                               start=(vc == 0), stop=(vc == ND - 1))
                    for d in range(ND):
                        nc.scalar.copy(ut_h[:, d * 512:(d + 1) * 512], pss[d][:])
                    return ut_h

                def z_chain(h, ut_h, j):
                    ps = mmps.tile([P, 512], F32, tag="mm")
                    for d in range(ND):
                        mm(ps[:],
                           xt[:, d * S + j * P: d * S + (j + 1) * P],
                           ut_h[:, d * 512:(d + 1) * 512],
                           start=(d == 0), stop=(d == ND - 1))
                    nc.scalar.copy(zres[:, j * D + h * 512: j * D + (h + 1) * 512], ps[:])

                ut0 = ut_compute(0)
                ut1 = ut_compute(1)
                # z halves interleaved per j: halves the xt-quarter arrival rate
                # the stream has to sustain
                for j in range(NS):
                    z_chain(0, ut0, j)
                    z_chain(1, ut1, j)

            # ---------------- Phase A2/A3: M then F (F resident) ----------------
            with ExitStack() as pa:
                mmps2 = pa.enter_context(tc.tile_pool(name="mmps2", bufs=6, space="PSUM"))
                mwork = pa.enter_context(tc.tile_pool(name="mwork", bufs=1))

                mres = mwork.tile([P, ND * D], MM)  # M d1-tile -> [:, d1*D + d2] = [d1-part, d2]

                # A2: M = wq.T @ wk
                for q in range(4):           # d1-pairs
                    pq = [mmps2.tile([P, 512], F32, name=f"mq{i}", tag="mm") for i in range(4)]
                    for ct in range(ND):
                        for dl in range(2):
                            for ch in range(2):
                                mm(pq[dl * 2 + ch][:],
                                   wqn[:, ct * D + (q * 2 + dl) * P: ct * D + (q * 2 + dl + 1) * P],
                                   wkn[:, ct * D + ch * 512: ct * D + (ch + 1) * 512],
                                   start=(ct == 0), stop=(ct == ND - 1))
                    for dl in range(2):
                        for ch in range(2):
                            d1 = q * 2 + dl
                            nc.scalar.copy(mres[:, d1 * D + ch * 512: d1 * D + (ch + 1) * 512],
                                           pq[dl * 2 + ch][:])

                # A3: F[d2,i] = sum_d1 M[d1,d2] xT[d1,i]  (UNSCALED; kept in SBUF)
                for d2 in range(ND):
                    pss = [mmps2.tile([P, 512], F32, name=f"fps{ic}", tag="mm") for ic in range(4)]
                    for d1 in range(ND):
                        for ic in range(4):
                            mm(pss[ic][:],
                               mres[:, d1 * D + d2 * P: d1 * D + (d2 + 1) * P],
                               xt[:, d1 * S + ic * 512: d1 * S + (ic + 1) * 512],
                               start=(d1 == 0), stop=(d1 == ND - 1))
                    for ic in range(4):
                        nc.scalar.copy(fres[:, d2 * S + ic * 512: d2 * S + (ic + 1) * 512],
                                       pss[ic][:])

        # ---------------- Phase B ----------------
        with ExitStack() as pb:
            scps = pb.enter_context(tc.tile_pool(name="scps", bufs=3, space="PSUM"))
            outps = pb.enter_context(tc.tile_pool(name="outps", bufs=3, space="PSUM"))
            miscps = pb.enter_context(tc.tile_pool(name="miscps", bufs=2, space="PSUM"))
            expp = pb.enter_context(tc.tile_pool(name="expp", bufs=18))
            outsb = pb.enter_context(tc.tile_pool(name="outsb", bufs=3))
            rsp = pb.enter_context(tc.tile_pool(name="rsp", bufs=2))
            rtp_pool = pb.enter_context(tc.tile_pool(name="rtp_pool", bufs=6))

            for sbi in range(NSB):
                # scoresT + exp per j-tile; DVE accumulates the j-partial
                # rowsums so PE only pays one 512-wide ones-matmul per sb
                ets = []
                rs_acc = rsp.tile([P, SB], R32, tag="ra")
                for j in range(NS):
                    sc = scps.tile([P, SB], F32, tag="sc")
                    for d2 in range(ND):
                        mm(sc[:],
                           xt[:, d2 * S + j * P: d2 * S + (j + 1) * P],
                           fres[:, d2 * S + sbi * SB: d2 * S + (sbi + 1) * SB],
                           start=(d2 == 0), stop=(d2 == ND - 1))
                    et = expp.tile([P, SB], MM, name=f"et{j}", tag="et")
                    nc.scalar.activation(et[:], sc[:], EXP, scale=SCALE)
                    ets.append(et)
                    if j == 0:
                        nc.vector.tensor_copy(rs_acc[:], et[:])
                    else:
                        nc.vector.tensor_add(rs_acc[:], rs_acc[:], et[:])

                def out_group(gi, recips):
                    it, ch = gi // 2, gi % 2
                    op = outps.tile([P, 512], F32, name=f"op{ch}", tag="op")
                    for j in range(NS):
                        mm(op[:],
                           ets[j][:, it * P:(it + 1) * P],
                           zres[:, j * D + ch * 512: j * D + (ch + 1) * 512],
                           start=(j == 0), stop=(j == NS - 1))
                    ob = outsb.tile([P, 512], DT, tag="ob")
                    nc.scalar.activation(ob[:], op[:], COPY, scale=recips[it][:, 0:1])
                    nc.sync.dma_start(
                        out=out_d[(sbi * NIT + it) * P:(sbi * NIT + it + 1) * P,
                                  ch * 512:(ch + 1) * 512],
                        in_=ob[:])

                # out-group 0 j-chain ramps while the last exps drain; PE then
                # does the rowsum matmul + tiny recip transposes, then the rest
                recips = [None] * NIT
                it, ch = 0, 0
                op0 = outps.tile([P, 512], F32, name="op0f", tag="op")
                for j in range(NS):
                    mm(op0[:],
                       ets[j][:, 0:P],
                       zres[:, j * D: j * D + 512],
                       start=(j == 0), stop=(j == NS - 1))

                rs = miscps.tile([1, SB], F32, tag="m")
                mm(rs[:], ones_r[:, 0:1], rs_acc[:], start=True, stop=True)
                rs_sb = rsp.tile([1, SB], DT, tag="rs")
                nc.vector.tensor_copy(rs_sb[:], rs[:])
                rc_sb = rsp.tile([1, SB], DT, tag="rc")
                nc.vector.reciprocal(rc_sb[:], rs_sb[:])
                for it2 in range(NIT):
                    tp = miscps.tile([P, 1], F32, name=f"rtp{it2}", tag="m")
                    nc.tensor.transpose(tp[:], rc_sb[:1, it2 * P:(it2 + 1) * P], ident_f32[:1, :1])
                    rt = rtp_pool.tile([P, 1], DT, name=f"rt{it2}", tag="rt")
                    nc.vector.tensor_copy(rt[:], tp[:])
                    recips[it2] = rt

                ob0 = outsb.tile([P, 512], DT, tag="ob")
                nc.scalar.activation(ob0[:], op0[:], COPY, scale=recips[0][:, 0:1])
                nc.sync.dma_start(
                    out=out_d[sbi * NIT * P:(sbi * NIT + 1) * P, 0:512],
                    in_=ob0[:])

                for gi in range(1, NIT * 2):
                    out_group(gi, recips)

    nc.compile()
    return nc


_NC_CACHE = None


def kernel(x, wq, wk, wv, wo):
    global _NC_CACHE
    if _NC_CACHE is None:
        _NC_CACHE = _build()
    nc = _NC_CACHE
    core_ids = list(range(N_CORES))
    wq16 = np.ascontiguousarray(wq, dtype=np.float16)
    wk16 = np.ascontiguousarray(wk, dtype=np.float16)
    wv16 = np.ascontiguousarray(wv, dtype=np.float16)
    wot16 = np.ascontiguousarray(wo.astype(np.float16).T)
    in_maps = []
    for b in range(N_CORES):
        in_maps.append({
            "xt": np.ascontiguousarray(x[b].astype(np.float16).T),
            "wq": wq16,
            "wk": wk16,
            "wv": wv16,
            "wot": wot16,
        })
    res = run_bass_kernel_spmd(nc, in_maps, core_ids)
    return np.stack([res.results[b]["out"] for b in range(N_CORES)], axis=0)


# revision 17
# speedup vs baseline: 1.1783x; 1.1783x over previous
"""Single-head encoder attention block on 8 Trainium2 NeuronCores.

Math (per batch element b):
    q = x @ wq.T ; k = x @ wk.T ; v = x @ wv.T
    scores = (q @ k.T) / sqrt(1024) ; attn = softmax(scores, -1)
    out = (attn @ v) @ wo.T

Sharding: data-parallel over batch - batch 8 maps 1:1 onto the 8 cores;
weights replicated. No collectives.

Per-core algorithm (matmul operands fp16; PSUM accumulation fp32; host
prepares device inputs: fp16 casts plus xT / woT layout):
  Two weight-product folds remove all operand transposes on device:
      scores = x (wq.T wk) x.T / 32            M  := wq.T @ wk
      attn @ v @ wo.T = attn @ x @ (wo wv).T   via UT[d,do] = sum_vc wv[vc,d] woT[vc,do]
  fp16 operands run every matmul at 1 cycle/row, halve input DMA
  (12MB/core), and let F = M @ xT stay fully resident in SBUF (no DRAM
  spill).  The 1/sqrt(dk)=1/32 scale is folded into the Exp activation.
  Rowsums accumulate on the Vector engine (tensor_add chain) so PE pays a
  single 512-wide ones-matmul per superblock.
  Phase A (DMA rings: scalar=xT; sync=woT; gpsimd/vector=wv,wk,wq):
    A0: UT h0 = wv-nat x woT-h0; Z h0 (xT quarters stream in)
    A1: UT h1, Z h1
    A2: M = wq.T @ wk ; A3: F = M @ xT (resident, unscaled)
  Phase B (per i-superblock of SB=512):
    scoresT[j,i] = sum_d2 xT[d2,j]*F[d2,i]; expT = exp(scoresT/32)
    out[i,do] = (sum_j expT[j,i-tile] * Z[j,do]) * recip[i]  (expT stationary);
    out-group 0 runs before the rowsum matmul so PE rides through the
    last exp's latency.
"""

import os
import sys

for _p in ("/opt/trn_rl_repo", "/root/.axon_site/_ro/trn_rl_repo"):
    if os.path.isdir(_p) and _p not in sys.path:
        sys.path.insert(0, _p)

import numpy as np
from contextlib import ExitStack

import concourse.bacc as bacc
import concourse.tile as tile
from concourse import mybir, masks
from concourse.bass_utils import run_bass_kernel_spmd

P = 128
S = 2048          # sequence length (per core)
D = 1024          # model dim = dk = dv
NS = S // P       # 16 seq tiles
ND = D // P       # 8 dim tiles
SB = 512          # i-superblock width (query columns per block)
NSB = S // SB     # 4 superblocks
NIT = SB // P     # 4 i-tiles per superblock
SCALE = 1.0 / 32.0  # 1/sqrt(1024)
N_CORES = 8

DT = mybir.dt.float32
MM = mybir.dt.float16
R32 = mybir.dt.float32r
F32 = mybir.dt.float32
EXP = mybir.ActivationFunctionType.Exp
COPY = mybir.ActivationFunctionType.Copy


def _build():
    nc = bacc.Bacc("TRN2", target_bir_lowering=False, debug=False, num_devices=N_CORES)

    xt_in = nc.dram_tensor("xt", [D, S], MM, kind="ExternalInput").ap()
    wq_in = nc.dram_tensor("wq", [D, D], MM, kind="ExternalInput").ap()
    wk_in = nc.dram_tensor("wk", [D, D], MM, kind="ExternalInput").ap()
    wv_in = nc.dram_tensor("wv", [D, D], MM, kind="ExternalInput").ap()
    wot_in = nc.dram_tensor("wot", [D, D], MM, kind="ExternalInput").ap()
    out_d = nc.dram_tensor("out", [S, D], DT, kind="ExternalOutput").ap()

    mm = nc.tensor.matmul

    with tile.TileContext(nc) as tc, ExitStack() as top:
        cst = top.enter_context(tc.tile_pool(name="cst", bufs=1))
        ident_f32 = cst.tile([P, P], DT)
        masks.make_identity(nc, ident_f32[:])
        ones_f32 = cst.tile([P, 1], DT)
        nc.gpsimd.memset(ones_f32[:], 1.0)
        ones_r = cst.tile([P, 1], R32)
        nc.vector.tensor_copy(ones_r[:], ones_f32[:])

        res1 = top.enter_context(tc.tile_pool(name="res1", bufs=1))
        xt = res1.tile([P, ND * S], MM)    # xT: tile d -> [:, d*S:(d+1)*S] = [d-part, s]
        res2 = top.enter_context(tc.tile_pool(name="res2", bufs=1))
        zres = res2.tile([P, NS * D], MM)  # Z: tile j -> [:, j*D:(j+1)*D] = [j-part, do]
        res3 = top.enter_context(tc.tile_pool(name="res3", bufs=1))
        fres = res3.tile([P, ND * S], MM)  # F: tile d2 -> [:, d2*S:(d2+1)*S] = [d2-part, i]

        with ExitStack() as pall:
            wgt = pall.enter_context(tc.tile_pool(name="wgt", bufs=1))

            wvn = wgt.tile([P, ND * D], MM)    # wv natural: vc-tile t -> [:, t*D:(t+1)*D]
            wkn = wgt.tile([P, ND * D], MM)    # wk natural
            wqn = wgt.tile([P, ND * D], MM)    # wq natural: ct-tile t -> [:, t*D + d1]
            wot = wgt.tile([P, ND * D], MM)    # woT: vc-tile t -> [:, t*D + do]

            # ---- front-load all input DMAs ----
            # UT h0 is PE's first work: wv rides two rings (gpsimd even /
            # scalar odd) while woT h0 rides sync, so the vc-major UT chains
            # consume tiles at arrival pace from ~9us on.
            for t in range(ND):
                nc.sync.dma_start(
                    out=wot[:, t * D: t * D + 512],
                    in_=wot_in[t * P:(t + 1) * P, 0:512])
            for t in range(0, ND, 2):
                nc.gpsimd.dma_start(out=wvn[:, t * D:(t + 1) * D], in_=wv_in[t * P:(t + 1) * P, :])
            for t in range(1, ND, 2):
                nc.scalar.dma_start(out=wvn[:, t * D:(t + 1) * D], in_=wv_in[t * P:(t + 1) * P, :])
            for t in range(ND):
                nc.sync.dma_start(
                    out=wot[:, t * D + 512: t * D + D],
                    in_=wot_in[t * P:(t + 1) * P, 512:D])
            # xT by s-quarters so Z chains can start after the first quarter
            for sq in range(4):
                for d in range(ND):
                    nc.scalar.dma_start(
                        out=xt[:, d * S + sq * 512: d * S + (sq + 1) * 512],
                        in_=xt_in[d * P:(d + 1) * P, sq * 512:(sq + 1) * 512])
            for t in range(ND):
                nc.gpsimd.dma_start(out=wkn[:, t * D:(t + 1) * D], in_=wk_in[t * P:(t + 1) * P, :])
            for t in range(ND):
                nc.sync.dma_start(out=wqn[:, t * D:(t + 1) * D], in_=wq_in[t * P:(t + 1) * P, :])

            # ---------------- Phase A0/A1: UT halves, Z halves ----------------
            with ExitStack() as pw:
                mmps = pw.enter_context(tc.tile_pool(name="mmps", bufs=8, space="PSUM"))
                hwork = pw.enter_context(tc.tile_pool(name="hwork", bufs=1))

                def ut_compute(h):
                    # vc-major accumulation across 8 PSUM banks: PE consumes
                    # each wv/woT tile as it lands instead of waiting for all
                    ut_h = hwork.tile([P, ND * 512], MM, name=f"uth{h}", tag="ut", bufs=2)
                    pss = [mmps.tile([P, 512], F32, name=f"utp{h}{d}", tag="mm")
                           for d in range(ND)]
                    for vc in range(ND):
                        for d in range(ND):
                            mm(pss[d][:],
                               wvn[:, vc * D + d * P: vc * D + (d + 1) * P],
                               wot[:, vc * D + h * 512: vc * D + (h + 1) * 512],
                               start=(vc == 0), stop=(vc == ND - 1))
                    for d in range(ND):
                        nc.scalar.copy(ut_h[:, d * 512:(d + 1) * 512], pss[d][:])
                    return ut_h

                def z_chain(h, ut_h, j):
                    ps = mmps.tile([P, 512], F32, tag="mm")
                    for d in range(ND):
                        mm(ps[:],
                           xt[:, d * S + j * P: d * S + (j + 1) * P],
                           ut_h[:, d * 512:(d + 1) * 512],
                           start=(d == 0), stop=(d == ND - 1))
                    nc.scalar.copy(zres[:, j * D + h * 512: j * D + (h + 1) * 512], ps[:])

                ut0 = ut_compute(0)
                ut1 = ut_compute(1)
                # z halves interleaved per j: halves the xt-quarter arrival rate
                # the stream has to sustain
                for j in range(NS):
                    z_chain(0, ut0, j)
                    z_chain(1, ut1, j)

            # ---------------- Phase A2/A3: M then F (F resident) ----------------
            with ExitStack() as pa:
                mmps2 = pa.enter_context(tc.tile_pool(name="mmps2", bufs=6, space="PSUM"))
                mwork = pa.enter_context(tc.tile_pool(name="mwork", bufs=1))

                mres = mwork.tile([P, ND * D], MM)  # M d1-tile -> [:, d1*D + d2] = [d1-part, d2]

                # A2: M = wq.T @ wk
                for q in range(4):           # d1-pairs
                    pq = [mmps2.tile([P, 512], F32, name=f"mq{i}", tag="mm") for i in range(4)]
                    for ct in range(ND):
                        for dl in range(2):
                            for ch in range(2):
                                mm(pq[dl * 2 + ch][:],
                                   wqn[:, ct * D + (q * 2 + dl) * P: ct * D + (q * 2 + dl + 1) * P],
                                   wkn[:, ct * D + ch * 512: ct * D + (ch + 1) * 512],
                                   start=(ct == 0), stop=(ct == ND - 1))
                    for dl in range(2):
                        for ch in range(2):
                            d1 = q * 2 + dl
                            nc.scalar.copy(mres[:, d1 * D + ch * 512: d1 * D + (ch + 1) * 512],
                                           pq[dl * 2 + ch][:])

                # A3: F[d2,i] = sum_d1 M[d1,d2] xT[d1,i]  (UNSCALED; kept in SBUF)
                for d2 in range(ND):
                    pss = [mmps2.tile([P, 512], F32, name=f"fps{ic}", tag="mm") for ic in range(4)]
                    for d1 in range(ND):
                        for ic in range(4):
                            mm(pss[ic][:],
                               mres[:, d1 * D + d2 * P: d1 * D + (d2 + 1) * P],
                               xt[:, d1 * S + ic * 512: d1 * S + (ic + 1) * 512],
                               start=(d1 == 0), stop=(d1 == ND - 1))
                    for ic in range(4):
                        nc.scalar.copy(fres[:, d2 * S + ic * 512: d2 * S + (ic + 1) * 512],
                                       pss[ic][:])

        # ---------------- Phase B ----------------
        with ExitStack() as pb:
            scps = pb.enter_context(tc.tile_pool(name="scps", bufs=3, space="PSUM"))
            outps = pb.enter_context(tc.tile_pool(name="outps", bufs=3, space="PSUM"))
            miscps = pb.enter_context(tc.tile_pool(name="miscps", bufs=2, space="PSUM"))
            expp = pb.enter_context(tc.tile_pool(name="expp", bufs=18))
            outsb = pb.enter_context(tc.tile_pool(name="outsb", bufs=3))
            rsp = pb.enter_context(tc.tile_pool(name="rsp", bufs=2))
            rtp_pool = pb.enter_context(tc.tile_pool(name="rtp_pool", bufs=6))

            for sbi in range(NSB):
                # scoresT + exp per j-tile; DVE accumulates the j-partial
                # rowsums so PE only pays one 512-wide ones-matmul per sb
                ets = []
                rs_acc = rsp.tile([P, SB], R32, tag="ra")
                for j in range(NS):
                    sc = scps.tile([P, SB], F32, tag="sc")
                    for d2 in range(ND):
                        mm(sc[:],
                           xt[:, d2 * S + j * P: d2 * S + (j + 1) * P],
                           fres[:, d2 * S + sbi * SB: d2 * S + (sbi + 1) * SB],
                           start=(d2 == 0), stop=(d2 == ND - 1))
                    et = expp.tile([P, SB], MM, name=f"et{j}", tag="et")
                    nc.scalar.activation(et[:], sc[:], EXP, scale=SCALE)
                    ets.append(et)
                    if j == 0:
                        nc.vector.tensor_copy(rs_acc[:], et[:])
                    else:
                        nc.vector.tensor_add(rs_acc[:], rs_acc[:], et[:])

                def out_group(gi, recips):
                    it, ch = gi // 2, gi % 2
                    op = outps.tile([P, 512], F32, name=f"op{ch}", tag="op")
                    for j in range(NS):
                        mm(op[:],
                           ets[j][:, it * P:(it + 1) * P],
                           zres[:, j * D + ch * 512: j * D + (ch + 1) * 512],
                           start=(j == 0), stop=(j == NS - 1))
                    ob = outsb.tile([P, 512], DT, tag="ob")
                    nc.scalar.activation(ob[:], op[:], COPY, scale=recips[it][:, 0:1])
                    nc.sync.dma_start(
                        out=out_d[(sbi * NIT + it) * P:(sbi * NIT + it + 1) * P,
                                  ch * 512:(ch + 1) * 512],
                        in_=ob[:])

                # out-group 0 j-chain ramps while the last exps drain; PE then
                # does the rowsum matmul + tiny recip transposes, then the rest
                recips = [None] * NIT
                it, ch = 0, 0
                op0 = outps.tile([P, 512], F32, name="op0f", tag="op")
                for j in range(NS):
                    mm(op0[:],
                       ets[j][:, 0:P],
                       zres[:, j * D: j * D + 512],
                       start=(j == 0), stop=(j == NS - 1))

                rs = miscps.tile([1, SB], F32, tag="m")
                mm(rs[:], ones_r[:, 0:1], rs_acc[:], start=True, stop=True)
                rs_sb = rsp.tile([1, SB], DT, tag="rs")
                nc.vector.tensor_copy(rs_sb[:], rs[:])
                rc_sb = rsp.tile([1, SB], DT, tag="rc")
                nc.vector.reciprocal(rc_sb[:], rs_sb[:])
                for it2 in range(NIT):
                    tp = miscps.tile([P, 1], F32, name=f"rtp{it2}", tag="m")
                    nc.tensor.transpose(tp[:], rc_sb[:1, it2 * P:(it2 + 1) * P], ident_f32[:1, :1])
                    rt = rtp_pool.tile([P, 1], DT, name=f"rt{it2}", tag="rt")
                    nc.vector.tensor_copy(rt[:], tp[:])
                    recips[it2] = rt

                ob0 = outsb.tile([P, 512], DT, tag="ob")
                nc.scalar.activation(ob0[:], op0[:], COPY, scale=recips[0][:, 0:1])
                nc.sync.dma_start(
                    out=out_d[sbi * NIT * P:(sbi * NIT + 1) * P, 0:512],
                    in_=ob0[:])

                for gi in range(1, NIT * 2):
                    out_group(gi, recips)

    nc.compile()
    return nc


_NC_CACHE = None


def kernel(x, wq, wk, wv, wo):
    global _NC_CACHE
    if _NC_CACHE is None:
        _NC_CACHE = _build()
    nc = _NC_CACHE
    core_ids = list(range(N_CORES))
    wq16 = np.ascontiguousarray(wq, dtype=np.float16)
    wk16 = np.ascontiguousarray(wk, dtype=np.float16)
    wv16 = np.ascontiguousarray(wv, dtype=np.float16)
    wot16 = np.ascontiguousarray(wo.astype(np.float16).T)
    in_maps = []
    for b in range(N_CORES):
        in_maps.append({
            "xt": np.ascontiguousarray(x[b].astype(np.float16).T),
            "wq": wq16,
            "wk": wk16,
            "wv": wv16,
            "wot": wot16,
        })
    res = run_bass_kernel_spmd(nc, in_maps, core_ids)
    return np.stack([res.results[b]["out"] for b in range(N_CORES)], axis=0)


# revision 19
# speedup vs baseline: 1.2284x; 1.0426x over previous
"""Single-head encoder attention block on 8 Trainium2 NeuronCores.

Math (per batch element b):
    q = x @ wq.T ; k = x @ wk.T ; v = x @ wv.T
    scores = (q @ k.T) / sqrt(1024) ; attn = softmax(scores, -1)
    out = (attn @ v) @ wo.T

Sharding: data-parallel over batch - batch 8 maps 1:1 onto the 8 cores;
weights replicated. No collectives.

Per-core algorithm (matmul operands fp16; PSUM accumulation fp32; host
prepares device inputs: fp16 casts plus xT / woT layout):
  Two weight-product folds remove all operand transposes on device:
      scores = x (wq.T wk) x.T / 32            M  := wq.T @ wk
      attn @ v @ wo.T = attn @ x @ (wo wv).T   via UT[d,do] = sum_vc wv[vc,d] woT[vc,do]
  fp16 operands run every matmul at 1 cycle/row, halve input DMA
  (12MB/core), and let F = M @ xT stay fully resident in SBUF (no DRAM
  spill).  The 1/sqrt(dk)=1/32 scale is folded into the Exp activation.
  Rowsums accumulate on the Vector engine (tensor_add chain) so PE pays a
  single 512-wide ones-matmul per superblock.
  Phase A (DMA rings: scalar=xT; sync=woT; gpsimd/vector=wv,wk,wq):
    A0: UT h0 = wv-nat x woT-h0; Z h0 (xT quarters stream in)
    A1: UT h1, Z h1
    A2: M = wq.T @ wk ; A3: F = M @ xT (resident, unscaled)
  Phase B (per i-superblock of SB=512):
    scoresT[j,i] = sum_d2 xT[d2,j]*F[d2,i]; expT = exp(scoresT/32)
    out[i,do] = (sum_j expT[j,i-tile] * Z[j,do]) * recip[i]  (expT stationary);
    out-group 0 runs before the rowsum matmul so PE rides through the
    last exp's latency.
"""

import os
import sys

for _p in ("/opt/trn_rl_repo", "/root/.axon_site/_ro/trn_rl_repo"):
    if os.path.isdir(_p) and _p not in sys.path:
        sys.path.insert(0, _p)

import numpy as np
from contextlib import ExitStack

import concourse.bacc as bacc
import concourse.tile as tile
from concourse import mybir, masks
from concourse.bass_utils import run_bass_kernel_spmd

P = 128
S = 2048          # sequence length (per core)
D = 1024          # model dim = dk = dv
NS = S // P       # 16 seq tiles
ND = D // P       # 8 dim tiles
SB = 512          # i-superblock width (query columns per block)
NSB = S // SB     # 4 superblocks
NIT = SB // P     # 4 i-tiles per superblock
SCALE = 1.0 / 32.0  # 1/sqrt(1024)
N_CORES = 8

DT = mybir.dt.float32
MM = mybir.dt.float16
R32 = mybir.dt.float32r
F32 = mybir.dt.float32
EXP = mybir.ActivationFunctionType.Exp
COPY = mybir.ActivationFunctionType.Copy


def _build():
    nc = bacc.Bacc("TRN2", target_bir_lowering=False, debug=False, num_devices=N_CORES)

    xt_in = nc.dram_tensor("xt", [D, S], MM, kind="ExternalInput").ap()
    wq_in = nc.dram_tensor("wq", [D, D], MM, kind="ExternalInput").ap()
    wk_in = nc.dram_tensor("wk", [D, D], MM, kind="ExternalInput").ap()
    wv_in = nc.dram_tensor("wv", [D, D], MM, kind="ExternalInput").ap()
    wot_in = nc.dram_tensor("wot", [D, D], MM, kind="ExternalInput").ap()
    out_d = nc.dram_tensor("out", [S, D], DT, kind="ExternalOutput").ap()

    mm = nc.tensor.matmul

    with tile.TileContext(nc) as tc, ExitStack() as top:
        cst = top.enter_context(tc.tile_pool(name="cst", bufs=1))
        ident_f32 = cst.tile([P, P], DT)
        masks.make_identity(nc, ident_f32[:])
        ones_f32 = cst.tile([P, 1], DT)
        nc.gpsimd.memset(ones_f32[:], 1.0)
        ones_r = cst.tile([P, 1], R32)
        nc.vector.tensor_copy(ones_r[:], ones_f32[:])

        res1 = top.enter_context(tc.tile_pool(name="res1", bufs=1))
        xt = res1.tile([P, ND * S], MM)    # xT: tile d -> [:, d*S:(d+1)*S] = [d-part, s]
        res2 = top.enter_context(tc.tile_pool(name="res2", bufs=1))
        zres = res2.tile([P, NS * D], MM)  # Z: tile j -> [:, j*D:(j+1)*D] = [j-part, do]
        res3 = top.enter_context(tc.tile_pool(name="res3", bufs=1))
        fres = res3.tile([P, ND * S], MM)  # F: tile d2 -> [:, d2*S:(d2+1)*S] = [d2-part, i]

        with ExitStack() as pall:
            wgt = pall.enter_context(tc.tile_pool(name="wgt", bufs=1))

            wvn = wgt.tile([P, ND * D], MM)    # wv natural: vc-tile t -> [:, t*D:(t+1)*D]
            wkn = wgt.tile([P, ND * D], MM)    # wk natural
            wqn = wgt.tile([P, ND * D], MM)    # wq natural: ct-tile t -> [:, t*D + d1]
            wot = wgt.tile([P, ND * D], MM)    # woT: vc-tile t -> [:, t*D + do]

            # ---- front-load all input DMAs ----
            # Rings balanced by need-time (each ring ~1/3 of aggregate BW):
            #   ut0 needs wv + woT-h0 (3MB) first, ut1 needs woT-h1 (~25us),
            #   z consumes xt by s-quarter (from ~40us), M needs wq/wk (~90us)
            def dma_xtq(ring, sq):
                for d in range(ND):
                    ring(out=xt[:, d * S + sq * 512: d * S + (sq + 1) * 512],
                         in_=xt_in[d * P:(d + 1) * P, sq * 512:(sq + 1) * 512])
            # sync: woT-h0, woT-h1[0:4], xt-q1, wq
            for t in range(ND):
                nc.sync.dma_start(
                    out=wot[:, t * D: t * D + 512],
                    in_=wot_in[t * P:(t + 1) * P, 0:512])
            # gpsimd: wv-even, woT-h1[4:8], xt-q2, wk
            for t in range(0, ND, 2):
                nc.gpsimd.dma_start(out=wvn[:, t * D:(t + 1) * D], in_=wv_in[t * P:(t + 1) * P, :])
            # scalar: wv-odd, xt-q0, xt-q3
            for t in range(1, ND, 2):
                nc.scalar.dma_start(out=wvn[:, t * D:(t + 1) * D], in_=wv_in[t * P:(t + 1) * P, :])
            for t in range(0, 4):
                nc.sync.dma_start(
                    out=wot[:, t * D + 512: t * D + D],
                    in_=wot_in[t * P:(t + 1) * P, 512:D])
            for t in range(4, ND):
                nc.gpsimd.dma_start(
                    out=wot[:, t * D + 512: t * D + D],
                    in_=wot_in[t * P:(t + 1) * P, 512:D])
            dma_xtq(nc.scalar.dma_start, 0)
            dma_xtq(nc.sync.dma_start, 1)
            dma_xtq(nc.gpsimd.dma_start, 2)
            dma_xtq(nc.scalar.dma_start, 3)
            for t in range(ND):
                nc.gpsimd.dma_start(out=wkn[:, t * D:(t + 1) * D], in_=wk_in[t * P:(t + 1) * P, :])
            for t in range(ND):
                nc.sync.dma_start(out=wqn[:, t * D:(t + 1) * D], in_=wq_in[t * P:(t + 1) * P, :])

            # ---------------- Phase A0/A1: UT halves, Z halves ----------------
            with ExitStack() as pw:
                mmps = pw.enter_context(tc.tile_pool(name="mmps", bufs=8, space="PSUM"))
                hwork = pw.enter_context(tc.tile_pool(name="hwork", bufs=1))

                def ut_compute(h):
                    # vc-major accumulation across 8 PSUM banks: PE consumes
                    # each wv/woT tile as it lands instead of waiting for all
                    ut_h = hwork.tile([P, ND * 512], MM, name=f"uth{h}", tag="ut", bufs=2)
                    pss = [mmps.tile([P, 512], F32, name=f"utp{h}{d}", tag="mm")
                           for d in range(ND)]
                    for vc in range(ND):
                        for d in range(ND):
                            mm(pss[d][:],
                               wvn[:, vc * D + d * P: vc * D + (d + 1) * P],
                               wot[:, vc * D + h * 512: vc * D + (h + 1) * 512],
                               start=(vc == 0), stop=(vc == ND - 1))
                    for d in range(ND):
                        nc.scalar.copy(ut_h[:, d * 512:(d + 1) * 512], pss[d][:])
                    return ut_h

                def z_chain(h, ut_h, j):
                    ps = mmps.tile([P, 512], F32, tag="mm")
                    for d in range(ND):
                        mm(ps[:],
                           xt[:, d * S + j * P: d * S + (j + 1) * P],
                           ut_h[:, d * 512:(d + 1) * 512],
                           start=(d == 0), stop=(d == ND - 1))
                    nc.scalar.copy(zres[:, j * D + h * 512: j * D + (h + 1) * 512], ps[:])

                ut0 = ut_compute(0)
                ut1 = ut_compute(1)
                # z halves interleaved per j: halves the xt-quarter arrival rate
                # the stream has to sustain
                for j in range(NS):
                    z_chain(0, ut0, j)
                    z_chain(1, ut1, j)

            # ---------------- Phase A2/A3: M then F (F resident) ----------------
            with ExitStack() as pa:
                mmps2 = pa.enter_context(tc.tile_pool(name="mmps2", bufs=6, space="PSUM"))
                mwork = pa.enter_context(tc.tile_pool(name="mwork", bufs=1))

                mres = mwork.tile([P, ND * D], MM)  # M d1-tile -> [:, d1*D + d2] = [d1-part, d2]

                # A2: M = wq.T @ wk
                for q in range(4):           # d1-pairs
                    pq = [mmps2.tile([P, 512], F32, name=f"mq{i}", tag="mm") for i in range(4)]
                    for ct in range(ND):
                        for dl in range(2):
                            for ch in range(2):
                                mm(pq[dl * 2 + ch][:],
                                   wqn[:, ct * D + (q * 2 + dl) * P: ct * D + (q * 2 + dl + 1) * P],
                                   wkn[:, ct * D + ch * 512: ct * D + (ch + 1) * 512],
                                   start=(ct == 0), stop=(ct == ND - 1))
                    for dl in range(2):
                        for ch in range(2):
                            d1 = q * 2 + dl
                            nc.scalar.copy(mres[:, d1 * D + ch * 512: d1 * D + (ch + 1) * 512],
                                           pq[dl * 2 + ch][:])

                # A3: F[d2,i] = sum_d1 M[d1,d2] xT[d1,i]  (UNSCALED; kept in SBUF)
                for d2 in range(ND):
                    pss = [mmps2.tile([P, 512], F32, name=f"fps{ic}", tag="mm") for ic in range(4)]
                    for d1 in range(ND):
                        for ic in range(4):
                            mm(pss[ic][:],
                               mres[:, d1 * D + d2 * P: d1 * D + (d2 + 1) * P],
                               xt[:, d1 * S + ic * 512: d1 * S + (ic + 1) * 512],
                               start=(d1 == 0), stop=(d1 == ND - 1))
                    for ic in range(4):
                        nc.scalar.copy(fres[:, d2 * S + ic * 512: d2 * S + (ic + 1) * 512],
                                       pss[ic][:])

        # ---------------- Phase B ----------------
        with ExitStack() as pb:
            scps = pb.enter_context(tc.tile_pool(name="scps", bufs=3, space="PSUM"))
            outps = pb.enter_context(tc.tile_pool(name="outps", bufs=3, space="PSUM"))
            miscps = pb.enter_context(tc.tile_pool(name="miscps", bufs=2, space="PSUM"))
            expp = pb.enter_context(tc.tile_pool(name="expp", bufs=18))
            outsb = pb.enter_context(tc.tile_pool(name="outsb", bufs=3))
            rsp = pb.enter_context(tc.tile_pool(name="rsp", bufs=2))
            rtp_pool = pb.enter_context(tc.tile_pool(name="rtp_pool", bufs=6))

            for sbi in range(NSB):
                # scoresT + exp per j-tile; DVE accumulates the j-partial
                # rowsums so PE only pays one 512-wide ones-matmul per sb
                ets = []
                rs_acc = rsp.tile([P, SB], R32, tag="ra")
                for j in range(NS):
                    sc = scps.tile([P, SB], F32, tag="sc")
                    for d2 in range(ND):
                        mm(sc[:],
                           xt[:, d2 * S + j * P: d2 * S + (j + 1) * P],
                           fres[:, d2 * S + sbi * SB: d2 * S + (sbi + 1) * SB],
                           start=(d2 == 0), stop=(d2 == ND - 1))
                    et = expp.tile([P, SB], MM, name=f"et{j}", tag="et")
                    nc.scalar.activation(et[:], sc[:], EXP, scale=SCALE)
                    ets.append(et)
                    if j == 0:
                        nc.vector.tensor_copy(rs_acc[:], et[:])
                    else:
                        nc.vector.tensor_add(rs_acc[:], rs_acc[:], et[:])

                def out_group(gi, recips):
                    it, ch = gi // 2, gi % 2
                    op = outps.tile([P, 512], F32, name=f"op{ch}", tag="op")
                    for j in range(NS):
                        mm(op[:],
                           ets[j][:, it * P:(it + 1) * P],
                           zres[:, j * D + ch * 512: j * D + (ch + 1) * 512],
                           start=(j == 0), stop=(j == NS - 1))
                    ob = outsb.tile([P, 512], DT, tag="ob")
                    nc.scalar.activation(ob[:], op[:], COPY, scale=recips[it][:, 0:1])
                    nc.sync.dma_start(
                        out=out_d[(sbi * NIT + it) * P:(sbi * NIT + it + 1) * P,
                                  ch * 512:(ch + 1) * 512],
                        in_=ob[:])

                # out-group 0 j-chain ramps while the last exps drain; the
                # rowsum matmul then feeds the DVE reciprocal, whose 3.3us
                # latency hides under out-group 1's j-chain
                recips = [None] * NIT
                op01 = []
                for ch in range(2):
                    op = outps.tile([P, 512], F32, name=f"op0f{ch}", tag="op")
                    for j in range(NS):
                        mm(op[:],
                           ets[j][:, 0:P],
                           zres[:, j * D + ch * 512: j * D + ch * 512 + 512],
                           start=(j == 0), stop=(j == NS - 1))
                    op01.append(op)
                    if ch == 0:
                        rs = miscps.tile([1, SB], F32, tag="m")
                        mm(rs[:], ones_r[:, 0:1], rs_acc[:], start=True, stop=True)
                        rs_sb = rsp.tile([1, SB], DT, tag="rs")
                        nc.vector.tensor_copy(rs_sb[:], rs[:])
                        rc_sb = rsp.tile([1, SB], DT, tag="rc")
                        nc.vector.reciprocal(rc_sb[:], rs_sb[:])
                for it2 in range(NIT):
                    tp = miscps.tile([P, 1], F32, name=f"rtp{it2}", tag="m")
                    nc.tensor.transpose(tp[:], rc_sb[:1, it2 * P:(it2 + 1) * P], ident_f32[:1, :1])
                    rt = rtp_pool.tile([P, 1], DT, name=f"rt{it2}", tag="rt")
                    nc.vector.tensor_copy(rt[:], tp[:])
                    recips[it2] = rt

                for ch in range(2):
                    ob0 = outsb.tile([P, 512], DT, tag="ob")
                    nc.scalar.activation(ob0[:], op01[ch][:], COPY, scale=recips[0][:, 0:1])
                    nc.sync.dma_start(
                        out=out_d[sbi * NIT * P:(sbi * NIT + 1) * P, ch * 512:(ch + 1) * 512],
                        in_=ob0[:])

                for gi in range(2, NIT * 2):
                    out_group(gi, recips)

    nc.compile()
    return nc


_NC_CACHE = None


def kernel(x, wq, wk, wv, wo):
    global _NC_CACHE
    if _NC_CACHE is None:
        _NC_CACHE = _build()
    nc = _NC_CACHE
    core_ids = list(range(N_CORES))
    wq16 = np.ascontiguousarray(wq, dtype=np.float16)
    wk16 = np.ascontiguousarray(wk, dtype=np.float16)
    wv16 = np.ascontiguousarray(wv, dtype=np.float16)
    wot16 = np.ascontiguousarray(wo.astype(np.float16).T)
    in_maps = []
    for b in range(N_CORES):
        in_maps.append({
            "xt": np.ascontiguousarray(x[b].astype(np.float16).T),
            "wq": wq16,
            "wk": wk16,
            "wv": wv16,
            "wot": wot16,
        })
    res = run_bass_kernel_spmd(nc, in_maps, core_ids)
    return np.stack([res.results[b]["out"] for b in range(N_CORES)], axis=0)


# revision 20
# speedup vs baseline: 1.2625x; 1.0277x over previous
"""Single-head encoder attention block on 8 Trainium2 NeuronCores.

Math (per batch element b):
    q = x @ wq.T ; k = x @ wk.T ; v = x @ wv.T
    scores = (q @ k.T) / sqrt(1024) ; attn = softmax(scores, -1)
    out = (attn @ v) @ wo.T

Sharding: data-parallel over batch - batch 8 maps 1:1 onto the 8 cores;
weights replicated. No collectives.

Per-core algorithm (matmul operands fp16; PSUM accumulation fp32; host
prepares device inputs: fp16 casts plus xT / woT layout):
  Two weight-product folds remove all operand transposes on device:
      scores = x (wq.T wk) x.T / 32            M  := wq.T @ wk
      attn @ v @ wo.T = attn @ x @ (wo wv).T   via UT[d,do] = sum_vc wv[vc,d] woT[vc,do]
  fp16 operands run every matmul at 1 cycle/row, halve input DMA
  (12MB/core), and let F = M @ xT stay fully resident in SBUF (no DRAM
  spill).  The 1/sqrt(dk)=1/32 scale is folded into the Exp activation.
  Rowsums accumulate on the Vector engine (tensor_add chain) so PE pays a
  single 512-wide ones-matmul per superblock.
  Phase A (DMA rings: scalar=xT; sync=woT; gpsimd/vector=wv,wk,wq):
    A0: UT h0 = wv-nat x woT-h0; Z h0 (xT quarters stream in)
    A1: UT h1, Z h1
    A2: M = wq.T @ wk ; A3: F = M @ xT (resident, unscaled)
  Phase B (per i-superblock of SB=512):
    scoresT[j,i] = sum_d2 xT[d2,j]*F[d2,i]; expT = exp(scoresT/32)
    out[i,do] = (sum_j expT[j,i-tile] * Z[j,do]) * recip[i]  (expT stationary);
    out-group 0 runs before the rowsum matmul so PE rides through the
    last exp's latency.
"""

import os
import sys

for _p in ("/opt/trn_rl_repo", "/root/.axon_site/_ro/trn_rl_repo"):
    if os.path.isdir(_p) and _p not in sys.path:
        sys.path.insert(0, _p)

import numpy as np
from contextlib import ExitStack

import concourse.bacc as bacc
import concourse.tile as tile
from concourse import mybir, masks
from concourse.bass_utils import run_bass_kernel_spmd

P = 128
S = 2048          # sequence length (per core)
D = 1024          # model dim = dk = dv
NS = S // P       # 16 seq tiles
ND = D // P       # 8 dim tiles
SB = 512          # i-superblock width (query columns per block)
NSB = S // SB     # 4 superblocks
NIT = SB // P     # 4 i-tiles per superblock
SCALE = 1.0 / 32.0  # 1/sqrt(1024)
N_CORES = 8

DT = mybir.dt.float32
MM = mybir.dt.float16
R32 = mybir.dt.float32r
F32 = mybir.dt.float32
EXP = mybir.ActivationFunctionType.Exp
COPY = mybir.ActivationFunctionType.Copy


def _build():
    nc = bacc.Bacc("TRN2", target_bir_lowering=False, debug=False, num_devices=N_CORES)

    xt_in = nc.dram_tensor("xt", [D, S], MM, kind="ExternalInput").ap()
    wq_in = nc.dram_tensor("wq", [D, D], MM, kind="ExternalInput").ap()
    wk_in = nc.dram_tensor("wk", [D, D], MM, kind="ExternalInput").ap()
    wv_in = nc.dram_tensor("wv", [D, D], MM, kind="ExternalInput").ap()
    wot_in = nc.dram_tensor("wot", [D, D], MM, kind="ExternalInput").ap()
    out_d = nc.dram_tensor("out", [S, D], DT, kind="ExternalOutput").ap()

    mm = nc.tensor.matmul

    with tile.TileContext(nc) as tc, ExitStack() as top:
        cst = top.enter_context(tc.tile_pool(name="cst", bufs=1))
        ident_f32 = cst.tile([P, P], DT)
        masks.make_identity(nc, ident_f32[:])
        ones_f32 = cst.tile([P, 1], DT)
        nc.gpsimd.memset(ones_f32[:], 1.0)
        ones_r = cst.tile([P, 1], R32)
        nc.vector.tensor_copy(ones_r[:], ones_f32[:])

        res1 = top.enter_context(tc.tile_pool(name="res1", bufs=1))
        xt = res1.tile([P, ND * S], MM)    # xT: tile d -> [:, d*S:(d+1)*S] = [d-part, s]
        res2 = top.enter_context(tc.tile_pool(name="res2", bufs=1))
        zres = res2.tile([P, NS * D], MM)  # Z: tile j -> [:, j*D:(j+1)*D] = [j-part, do]
        res3 = top.enter_context(tc.tile_pool(name="res3", bufs=1))
        fres = res3.tile([P, ND * S], MM)  # F: tile d2 -> [:, d2*S:(d2+1)*S] = [d2-part, i]

        with ExitStack() as pall:
            wgt = pall.enter_context(tc.tile_pool(name="wgt", bufs=1))

            wvn = wgt.tile([P, ND * D], MM)    # wv natural: vc-tile t -> [:, t*D:(t+1)*D]
            wkn = wgt.tile([P, ND * D], MM)    # wk natural
            wqn = wgt.tile([P, ND * D], MM)    # wq natural: ct-tile t -> [:, t*D + d1]
            wot = wgt.tile([P, ND * D], MM)    # woT: vc-tile t -> [:, t*D + do]

            # ---- front-load all input DMAs ----
            # Rings balanced by need-time (each ring ~1/3 of aggregate BW):
            #   ut0 needs wv + woT-h0 (3MB) first, ut1 needs woT-h1 (~25us),
            #   z consumes xt by s-quarter (from ~40us), M needs wq/wk (~90us)
            def dma_xtq(ring, sq):
                for d in range(ND):
                    ring(out=xt[:, d * S + sq * 512: d * S + (sq + 1) * 512],
                         in_=xt_in[d * P:(d + 1) * P, sq * 512:(sq + 1) * 512])
            # sync: woT-h0, woT-h1[0:4], xt-q1, wq
            for t in range(ND):
                nc.sync.dma_start(
                    out=wot[:, t * D: t * D + 512],
                    in_=wot_in[t * P:(t + 1) * P, 0:512])
            # gpsimd: wv-even, woT-h1[4:8], xt-q2, wk
            for t in range(0, ND, 2):
                nc.gpsimd.dma_start(out=wvn[:, t * D:(t + 1) * D], in_=wv_in[t * P:(t + 1) * P, :])
            # scalar: wv-odd, xt-q0, xt-q3
            for t in range(1, ND, 2):
                nc.scalar.dma_start(out=wvn[:, t * D:(t + 1) * D], in_=wv_in[t * P:(t + 1) * P, :])
            for t in range(0, 4):
                nc.sync.dma_start(
                    out=wot[:, t * D + 512: t * D + D],
                    in_=wot_in[t * P:(t + 1) * P, 512:D])
            for t in range(4, ND):
                nc.gpsimd.dma_start(
                    out=wot[:, t * D + 512: t * D + D],
                    in_=wot_in[t * P:(t + 1) * P, 512:D])
            dma_xtq(nc.scalar.dma_start, 0)
            dma_xtq(nc.sync.dma_start, 1)
            dma_xtq(nc.gpsimd.dma_start, 2)
            dma_xtq(nc.gpsimd.dma_start, 3)
            for t in range(ND):
                nc.gpsimd.dma_start(out=wkn[:, t * D:(t + 1) * D], in_=wk_in[t * P:(t + 1) * P, :])
            for t in range(ND):
                nc.sync.dma_start(out=wqn[:, t * D:(t + 1) * D], in_=wq_in[t * P:(t + 1) * P, :])

            # ---------------- Phase A0/A1: UT halves, Z halves ----------------
            with ExitStack() as pw:
                mmps = pw.enter_context(tc.tile_pool(name="mmps", bufs=8, space="PSUM"))
                hwork = pw.enter_context(tc.tile_pool(name="hwork", bufs=1))

                def ut_compute(h):
                    # vc-major accumulation across 8 PSUM banks: PE consumes
                    # each wv/woT tile as it lands instead of waiting for all
                    ut_h = hwork.tile([P, ND * 512], MM, name=f"uth{h}", tag="ut", bufs=2)
                    pss = [mmps.tile([P, 512], F32, name=f"utp{h}{d}", tag="mm")
                           for d in range(ND)]
                    for vc in range(ND):
                        for d in range(ND):
                            mm(pss[d][:],
                               wvn[:, vc * D + d * P: vc * D + (d + 1) * P],
                               wot[:, vc * D + h * 512: vc * D + (h + 1) * 512],
                               start=(vc == 0), stop=(vc == ND - 1))
                    for d in range(ND):
                        nc.vector.tensor_copy(ut_h[:, d * 512:(d + 1) * 512], pss[d][:])
                    return ut_h

                def z_chain(h, ut_h, j):
                    ps = mmps.tile([P, 512], F32, tag="mm")
                    for d in range(ND):
                        mm(ps[:],
                           xt[:, d * S + j * P: d * S + (j + 1) * P],
                           ut_h[:, d * 512:(d + 1) * 512],
                           start=(d == 0), stop=(d == ND - 1))
                    nc.vector.tensor_copy(zres[:, j * D + h * 512: j * D + (h + 1) * 512], ps[:])

                ut0 = ut_compute(0)
                ut1 = ut_compute(1)
                # z halves interleaved per j: halves the xt-quarter arrival rate
                # the stream has to sustain
                for j in range(NS):
                    z_chain(0, ut0, j)
                    z_chain(1, ut1, j)

            # ---------------- Phase A2/A3: M then F (F resident) ----------------
            with ExitStack() as pa:
                mmps2 = pa.enter_context(tc.tile_pool(name="mmps2", bufs=6, space="PSUM"))
                mwork = pa.enter_context(tc.tile_pool(name="mwork", bufs=1))

                mres = mwork.tile([P, ND * D], MM)  # M d1-tile -> [:, d1*D + d2] = [d1-part, d2]

                # A2: M = wq.T @ wk
                for q in range(4):           # d1-pairs
                    pq = [mmps2.tile([P, 512], F32, name=f"mq{i}", tag="mm") for i in range(4)]
                    for ct in range(ND):
                        for dl in range(2):
                            for ch in range(2):
                                mm(pq[dl * 2 + ch][:],
                                   wqn[:, ct * D + (q * 2 + dl) * P: ct * D + (q * 2 + dl + 1) * P],
                                   wkn[:, ct * D + ch * 512: ct * D + (ch + 1) * 512],
                                   start=(ct == 0), stop=(ct == ND - 1))
                    for dl in range(2):
                        for ch in range(2):
                            d1 = q * 2 + dl
                            nc.vector.tensor_copy(mres[:, d1 * D + ch * 512: d1 * D + (ch + 1) * 512],
                                                  pq[dl * 2 + ch][:])

                # A3: F[d2,i] = sum_d1 M[d1,d2] xT[d1,i]  (UNSCALED; kept in SBUF)
                for d2 in range(ND):
                    pss = [mmps2.tile([P, 512], F32, name=f"fps{ic}", tag="mm") for ic in range(4)]
                    for d1 in range(ND):
                        for ic in range(4):
                            mm(pss[ic][:],
                               mres[:, d1 * D + d2 * P: d1 * D + (d2 + 1) * P],
                               xt[:, d1 * S + ic * 512: d1 * S + (ic + 1) * 512],
                               start=(d1 == 0), stop=(d1 == ND - 1))
                    for ic in range(4):
                        nc.vector.tensor_copy(fres[:, d2 * S + ic * 512: d2 * S + (ic + 1) * 512],
                                              pss[ic][:])

        # ---------------- Phase B ----------------
        with ExitStack() as pb:
            scps = pb.enter_context(tc.tile_pool(name="scps", bufs=3, space="PSUM"))
            outps = pb.enter_context(tc.tile_pool(name="outps", bufs=3, space="PSUM"))
            miscps = pb.enter_context(tc.tile_pool(name="miscps", bufs=2, space="PSUM"))
            expp = pb.enter_context(tc.tile_pool(name="expp", bufs=18))
            outsb = pb.enter_context(tc.tile_pool(name="outsb", bufs=3))
            rsp = pb.enter_context(tc.tile_pool(name="rsp", bufs=2))
            rtp_pool = pb.enter_context(tc.tile_pool(name="rtp_pool", bufs=6))

            for sbi in range(NSB):
                # scoresT + exp per j-tile; DVE accumulates the j-partial
                # rowsums so PE only pays one 512-wide ones-matmul per sb
                ets = []
                rs_acc = rsp.tile([P, SB], R32, tag="ra")
                for j in range(NS):
                    sc = scps.tile([P, SB], F32, tag="sc")
                    for d2 in range(ND):
                        mm(sc[:],
                           xt[:, d2 * S + j * P: d2 * S + (j + 1) * P],
                           fres[:, d2 * S + sbi * SB: d2 * S + (sbi + 1) * SB],
                           start=(d2 == 0), stop=(d2 == ND - 1))
                    et = expp.tile([P, SB], MM, name=f"et{j}", tag="et")
                    nc.scalar.activation(et[:], sc[:], EXP, scale=SCALE)
                    ets.append(et)
                    if j == 0:
                        nc.vector.tensor_copy(rs_acc[:], et[:])
                    else:
                        nc.vector.tensor_add(rs_acc[:], rs_acc[:], et[:])

                def out_group(gi, recips):
                    it, ch = gi // 2, gi % 2
                    op = outps.tile([P, 512], F32, name=f"op{ch}", tag="op")
                    for j in range(NS):
                        mm(op[:],
                           ets[j][:, it * P:(it + 1) * P],
                           zres[:, j * D + ch * 512: j * D + (ch + 1) * 512],
                           start=(j == 0), stop=(j == NS - 1))
                    ob = outsb.tile([P, 512], DT, tag="ob")
                    nc.scalar.activation(ob[:], op[:], COPY, scale=recips[it][:, 0:1])
                    nc.sync.dma_start(
                        out=out_d[(sbi * NIT + it) * P:(sbi * NIT + it + 1) * P,
                                  ch * 512:(ch + 1) * 512],
                        in_=ob[:])

                # out-group 0 j-chain ramps while the last exps drain; the
                # rowsum matmul then feeds the DVE reciprocal, whose 3.3us
                # latency hides under out-group 1's j-chain
                recips = [None] * NIT
                op01 = []
                for ch in range(2):
                    op = outps.tile([P, 512], F32, name=f"op0f{ch}", tag="op")
                    for j in range(NS):
                        mm(op[:],
                           ets[j][:, 0:P],
                           zres[:, j * D + ch * 512: j * D + ch * 512 + 512],
                           start=(j == 0), stop=(j == NS - 1))
                    op01.append(op)
                    if ch == 0:
                        rs = miscps.tile([1, SB], F32, tag="m")
                        mm(rs[:], ones_r[:, 0:1], rs_acc[:], start=True, stop=True)
                        rs_sb = rsp.tile([1, SB], DT, tag="rs")
                        nc.vector.tensor_copy(rs_sb[:], rs[:])
                        rc_sb = rsp.tile([1, SB], DT, tag="rc")
                        nc.vector.reciprocal(rc_sb[:], rs_sb[:])
                for it2 in range(NIT):
                    tp = miscps.tile([P, 1], F32, name=f"rtp{it2}", tag="m")
                    nc.tensor.transpose(tp[:], rc_sb[:1, it2 * P:(it2 + 1) * P], ident_f32[:1, :1])
                    rt = rtp_pool.tile([P, 1], DT, name=f"rt{it2}", tag="rt")
                    nc.vector.tensor_copy(rt[:], tp[:])
                    recips[it2] = rt

                for ch in range(2):
                    ob0 = outsb.tile([P, 512], DT, tag="ob")
                    nc.scalar.activation(ob0[:], op01[ch][:], COPY, scale=recips[0][:, 0:1])
                    nc.sync.dma_start(
                        out=out_d[sbi * NIT * P:(sbi * NIT + 1) * P, ch * 512:(ch + 1) * 512],
                        in_=ob0[:])

                for gi in range(2, NIT * 2):
                    out_group(gi, recips)

    nc.compile()
    return nc


_NC_CACHE = None


def kernel(x, wq, wk, wv, wo):
    global _NC_CACHE
    if _NC_CACHE is None:
        _NC_CACHE = _build()
    nc = _NC_CACHE
    core_ids = list(range(N_CORES))
    wq16 = np.ascontiguousarray(wq, dtype=np.float16)
    wk16 = np.ascontiguousarray(wk, dtype=np.float16)
    wv16 = np.ascontiguousarray(wv, dtype=np.float16)
    wot16 = np.ascontiguousarray(wo.astype(np.float16).T)
    in_maps = []
    for b in range(N_CORES):
        in_maps.append({
            "xt": np.ascontiguousarray(x[b].astype(np.float16).T),
            "wq": wq16,
            "wk": wk16,
            "wv": wv16,
            "wot": wot16,
        })
    res = run_bass_kernel_spmd(nc, in_maps, core_ids)
    return np.stack([res.results[b]["out"] for b in range(N_CORES)], axis=0)


# revision 21
# speedup vs baseline: 1.2694x; 1.0055x over previous
"""Single-head encoder attention block on 8 Trainium2 NeuronCores.

Math (per batch element b):
    q = x @ wq.T ; k = x @ wk.T ; v = x @ wv.T
    scores = (q @ k.T) / sqrt(1024) ; attn = softmax(scores, -1)
    out = (attn @ v) @ wo.T

Sharding: data-parallel over batch - batch 8 maps 1:1 onto the 8 cores;
weights replicated. No collectives.

Per-core algorithm (matmul operands fp16; PSUM accumulation fp32; host
prepares device inputs: fp16 casts plus xT / woT layout):
  Two weight-product folds remove all operand transposes on device:
      scores = x (wq.T wk) x.T / 32            M  := wq.T @ wk
      attn @ v @ wo.T = attn @ x @ (wo wv).T   via UT[d,do] = sum_vc wv[vc,d] woT[vc,do]
  fp16 operands run every matmul at 1 cycle/row, halve input DMA
  (12MB/core), and let F = M @ xT stay fully resident in SBUF (no DRAM
  spill).  The 1/sqrt(dk)=1/32 scale is folded into the Exp activation.
  Rowsums accumulate on the Vector engine (tensor_add chain) so PE pays a
  single 512-wide ones-matmul per superblock.
  Phase A (DMA rings: scalar=xT; sync=woT; gpsimd/vector=wv,wk,wq):
    A0: UT h0 = wv-nat x woT-h0; Z h0 (xT quarters stream in)
    A1: UT h1, Z h1
    A2: M = wq.T @ wk ; A3: F = M @ xT (resident, unscaled)
  Phase B (per i-superblock of SB=512):
    scoresT[j,i] = sum_d2 xT[d2,j]*F[d2,i]; expT = exp(scoresT/32)
    out[i,do] = (sum_j expT[j,i-tile] * Z[j,do]) * recip[i]  (expT stationary);
    out-group 0 runs before the rowsum matmul so PE rides through the
    last exp's latency.
"""

import os
import sys

for _p in ("/opt/trn_rl_repo", "/root/.axon_site/_ro/trn_rl_repo"):
    if os.path.isdir(_p) and _p not in sys.path:
        sys.path.insert(0, _p)

import numpy as np
from contextlib import ExitStack

import concourse.bacc as bacc
import concourse.tile as tile
from concourse import mybir, masks
from concourse.bass_utils import run_bass_kernel_spmd

P = 128
S = 2048          # sequence length (per core)
D = 1024          # model dim = dk = dv
NS = S // P       # 16 seq tiles
ND = D // P       # 8 dim tiles
SB = 512          # i-superblock width (query columns per block)
NSB = S // SB     # 4 superblocks
NIT = SB // P     # 4 i-tiles per superblock
SCALE = 1.0 / 32.0  # 1/sqrt(1024)
N_CORES = 8

DT = mybir.dt.float32
MM = mybir.dt.float16
R32 = mybir.dt.float32r
F32 = mybir.dt.float32
EXP = mybir.ActivationFunctionType.Exp
COPY = mybir.ActivationFunctionType.Copy


def _build():
    nc = bacc.Bacc("TRN2", target_bir_lowering=False, debug=False, num_devices=N_CORES)

    xt_in = nc.dram_tensor("xt", [D, S], MM, kind="ExternalInput").ap()
    wq_in = nc.dram_tensor("wq", [D, D], MM, kind="ExternalInput").ap()
    wk_in = nc.dram_tensor("wk", [D, D], MM, kind="ExternalInput").ap()
    wv_in = nc.dram_tensor("wv", [D, D], MM, kind="ExternalInput").ap()
    wot_in = nc.dram_tensor("wot", [D, D], MM, kind="ExternalInput").ap()
    out_d = nc.dram_tensor("out", [S, D], DT, kind="ExternalOutput").ap()

    mm = nc.tensor.matmul

    with tile.TileContext(nc) as tc, ExitStack() as top:
        cst = top.enter_context(tc.tile_pool(name="cst", bufs=1))
        ident_f32 = cst.tile([P, P], DT)
        masks.make_identity(nc, ident_f32[:])
        ones_f32 = cst.tile([P, 1], DT)
        nc.gpsimd.memset(ones_f32[:], 1.0)
        ones_r = cst.tile([P, 1], R32)
        nc.vector.tensor_copy(ones_r[:], ones_f32[:])

        res1 = top.enter_context(tc.tile_pool(name="res1", bufs=1))
        xt = res1.tile([P, ND * S], MM)    # xT: tile d -> [:, d*S:(d+1)*S] = [d-part, s]
        res2 = top.enter_context(tc.tile_pool(name="res2", bufs=1))
        zres = res2.tile([P, NS * D], MM)  # Z: tile j -> [:, j*D:(j+1)*D] = [j-part, do]
        res3 = top.enter_context(tc.tile_pool(name="res3", bufs=1))
        fres = res3.tile([P, ND * S], MM)  # F: tile d2 -> [:, d2*S:(d2+1)*S] = [d2-part, i]

        with ExitStack() as pall:
            wgt = pall.enter_context(tc.tile_pool(name="wgt", bufs=1))

            wvn = wgt.tile([P, ND * D], MM)    # wv natural: vc-tile t -> [:, t*D:(t+1)*D]
            wkn = wgt.tile([P, ND * D], MM)    # wk natural
            wqn = wgt.tile([P, ND * D], MM)    # wq natural: ct-tile t -> [:, t*D + d1]
            wot = wgt.tile([P, ND * D], MM)    # woT: vc-tile t -> [:, t*D + do]

            # ---- front-load all input DMAs ----
            # Rings balanced by need-time (each ring ~1/3 of aggregate BW):
            #   ut0 needs wv + woT-h0 (3MB) first, ut1 needs woT-h1 (~25us),
            #   z consumes xt by s-quarter (from ~40us), M needs wq/wk (~90us)
            def dma_xtq(ring, sq):
                for d in range(ND):
                    ring(out=xt[:, d * S + sq * 512: d * S + (sq + 1) * 512],
                         in_=xt_in[d * P:(d + 1) * P, sq * 512:(sq + 1) * 512])
            # sync: woT-h0, woT-h1[0:4], xt-q1, wq
            for t in range(ND):
                nc.sync.dma_start(
                    out=wot[:, t * D: t * D + 512],
                    in_=wot_in[t * P:(t + 1) * P, 0:512])
            # gpsimd: wv-even, woT-h1[4:8], xt-q2, wk
            for t in range(0, ND, 2):
                nc.gpsimd.dma_start(out=wvn[:, t * D:(t + 1) * D], in_=wv_in[t * P:(t + 1) * P, :])
            # scalar: wv-odd, xt-q0, xt-q3
            for t in range(1, ND, 2):
                nc.scalar.dma_start(out=wvn[:, t * D:(t + 1) * D], in_=wv_in[t * P:(t + 1) * P, :])
            for t in range(0, 4):
                nc.sync.dma_start(
                    out=wot[:, t * D + 512: t * D + D],
                    in_=wot_in[t * P:(t + 1) * P, 512:D])
            for t in range(4, ND):
                nc.scalar.dma_start(
                    out=wot[:, t * D + 512: t * D + D],
                    in_=wot_in[t * P:(t + 1) * P, 512:D])
            dma_xtq(nc.scalar.dma_start, 0)
            dma_xtq(nc.sync.dma_start, 1)
            dma_xtq(nc.gpsimd.dma_start, 2)
            dma_xtq(nc.gpsimd.dma_start, 3)
            for t in range(ND):
                nc.gpsimd.dma_start(out=wkn[:, t * D:(t + 1) * D], in_=wk_in[t * P:(t + 1) * P, :])
            for t in range(ND):
                nc.sync.dma_start(out=wqn[:, t * D:(t + 1) * D], in_=wq_in[t * P:(t + 1) * P, :])

            # ---------------- Phase A0/A1: UT halves, Z halves ----------------
            with ExitStack() as pw:
                mmps = pw.enter_context(tc.tile_pool(name="mmps", bufs=8, space="PSUM"))
                hwork = pw.enter_context(tc.tile_pool(name="hwork", bufs=1))

                def ut_compute(h):
                    # vc-major accumulation across 8 PSUM banks: PE consumes
                    # each wv/woT tile as it lands instead of waiting for all
                    ut_h = hwork.tile([P, ND * 512], MM, name=f"uth{h}", tag="ut", bufs=2)
                    pss = [mmps.tile([P, 512], F32, name=f"utp{h}{d}", tag="mm")
                           for d in range(ND)]
                    for vc in range(ND):
                        for d in range(ND):
                            mm(pss[d][:],
                               wvn[:, vc * D + d * P: vc * D + (d + 1) * P],
                               wot[:, vc * D + h * 512: vc * D + (h + 1) * 512],
                               start=(vc == 0), stop=(vc == ND - 1))
                    for d in range(ND):
                        nc.vector.tensor_copy(ut_h[:, d * 512:(d + 1) * 512], pss[d][:])
                    return ut_h

                def z_chain(h, ut_h, j):
                    ps = mmps.tile([P, 512], F32, tag="mm")
                    for d in range(ND):
                        mm(ps[:],
                           xt[:, d * S + j * P: d * S + (j + 1) * P],
                           ut_h[:, d * 512:(d + 1) * 512],
                           start=(d == 0), stop=(d == ND - 1))
                    nc.vector.tensor_copy(zres[:, j * D + h * 512: j * D + (h + 1) * 512], ps[:])

                ut0 = ut_compute(0)
                ut1 = ut_compute(1)
                # z halves interleaved per j: halves the xt-quarter arrival rate
                # the stream has to sustain
                for j in range(NS):
                    z_chain(0, ut0, j)
                    z_chain(1, ut1, j)

            # ---------------- Phase A2/A3: M then F (F resident) ----------------
            with ExitStack() as pa:
                mmps2 = pa.enter_context(tc.tile_pool(name="mmps2", bufs=6, space="PSUM"))
                mwork = pa.enter_context(tc.tile_pool(name="mwork", bufs=1))

                mres = mwork.tile([P, ND * D], MM)  # M d1-tile -> [:, d1*D + d2] = [d1-part, d2]

                # A2: M = wq.T @ wk
                for q in range(4):           # d1-pairs
                    pq = [mmps2.tile([P, 512], F32, name=f"mq{i}", tag="mm") for i in range(4)]
                    for ct in range(ND):
                        for dl in range(2):
                            for ch in range(2):
                                mm(pq[dl * 2 + ch][:],
                                   wqn[:, ct * D + (q * 2 + dl) * P: ct * D + (q * 2 + dl + 1) * P],
                                   wkn[:, ct * D + ch * 512: ct * D + (ch + 1) * 512],
                                   start=(ct == 0), stop=(ct == ND - 1))
                    for dl in range(2):
                        for ch in range(2):
                            d1 = q * 2 + dl
                            nc.vector.tensor_copy(mres[:, d1 * D + ch * 512: d1 * D + (ch + 1) * 512],
                                                  pq[dl * 2 + ch][:])

                # A3: F[d2,i] = sum_d1 M[d1,d2] xT[d1,i]  (UNSCALED; kept in SBUF)
                for d2 in range(ND):
                    pss = [mmps2.tile([P, 512], F32, name=f"fps{ic}", tag="mm") for ic in range(4)]
                    for d1 in range(ND):
                        for ic in range(4):
                            mm(pss[ic][:],
                               mres[:, d1 * D + d2 * P: d1 * D + (d2 + 1) * P],
                               xt[:, d1 * S + ic * 512: d1 * S + (ic + 1) * 512],
                               start=(d1 == 0), stop=(d1 == ND - 1))
                    for ic in range(4):
                        nc.vector.tensor_copy(fres[:, d2 * S + ic * 512: d2 * S + (ic + 1) * 512],
                                              pss[ic][:])

        # ---------------- Phase B ----------------
        with ExitStack() as pb:
            scps = pb.enter_context(tc.tile_pool(name="scps", bufs=3, space="PSUM"))
            outps = pb.enter_context(tc.tile_pool(name="outps", bufs=3, space="PSUM"))
            miscps = pb.enter_context(tc.tile_pool(name="miscps", bufs=2, space="PSUM"))
            expp = pb.enter_context(tc.tile_pool(name="expp", bufs=18))
            outsb = pb.enter_context(tc.tile_pool(name="outsb", bufs=3))
            rsp = pb.enter_context(tc.tile_pool(name="rsp", bufs=2))
            rtp_pool = pb.enter_context(tc.tile_pool(name="rtp_pool", bufs=6))

            for sbi in range(NSB):
                # scoresT + exp per j-tile; DVE accumulates the j-partial
                # rowsums so PE only pays one 512-wide ones-matmul per sb
                ets = []
                rs_acc = rsp.tile([P, SB], R32, tag="ra")
                for j in range(NS):
                    sc = scps.tile([P, SB], F32, tag="sc")
                    for d2 in range(ND):
                        mm(sc[:],
                           xt[:, d2 * S + j * P: d2 * S + (j + 1) * P],
                           fres[:, d2 * S + sbi * SB: d2 * S + (sbi + 1) * SB],
                           start=(d2 == 0), stop=(d2 == ND - 1))
                    et = expp.tile([P, SB], MM, name=f"et{j}", tag="et")
                    nc.scalar.activation(et[:], sc[:], EXP, scale=SCALE)
                    ets.append(et)
                    if j == 0:
                        nc.vector.tensor_copy(rs_acc[:], et[:])
                    else:
                        nc.vector.tensor_add(rs_acc[:], rs_acc[:], et[:])

                def out_group(gi, recips):
                    it, ch = gi // 2, gi % 2
                    op = outps.tile([P, 512], F32, name=f"op{ch}", tag="op")
                    for j in range(NS):
                        mm(op[:],
                           ets[j][:, it * P:(it + 1) * P],
                           zres[:, j * D + ch * 512: j * D + (ch + 1) * 512],
                           start=(j == 0), stop=(j == NS - 1))
                    ob = outsb.tile([P, 512], DT, tag="ob")
                    nc.scalar.activation(ob[:], op[:], COPY, scale=recips[it][:, 0:1])
                    nc.sync.dma_start(
                        out=out_d[(sbi * NIT + it) * P:(sbi * NIT + it + 1) * P,
                                  ch * 512:(ch + 1) * 512],
                        in_=ob[:])

                # out-group 0 j-chain ramps while the last exps drain; the
                # rowsum matmul then feeds the DVE reciprocal, whose 3.3us
                # latency hides under out-group 1's j-chain
                recips = [None] * NIT
                op01 = []
                for ch in range(2):
                    op = outps.tile([P, 512], F32, name=f"op0f{ch}", tag="op")
                    for j in range(NS):
                        mm(op[:],
                           ets[j][:, 0:P],
                           zres[:, j * D + ch * 512: j * D + ch * 512 + 512],
                           start=(j == 0), stop=(j == NS - 1))
                    op01.append(op)
                    if ch == 0:
                        rs = miscps.tile([1, SB], F32, tag="m")
                        mm(rs[:], ones_r[:, 0:1], rs_acc[:], start=True, stop=True)
                        rs_sb = rsp.tile([1, SB], DT, tag="rs")
                        nc.vector.tensor_copy(rs_sb[:], rs[:])
                        rc_sb = rsp.tile([1, SB], DT, tag="rc")
                        nc.vector.reciprocal(rc_sb[:], rs_sb[:])
                for it2 in range(NIT):
                    tp = miscps.tile([P, 1], F32, name=f"rtp{it2}", tag="m")
                    nc.tensor.transpose(tp[:], rc_sb[:1, it2 * P:(it2 + 1) * P], ident_f32[:1, :1])
                    rt = rtp_pool.tile([P, 1], DT, name=f"rt{it2}", tag="rt")
                    nc.vector.tensor_copy(rt[:], tp[:])
                    recips[it2] = rt

                for ch in range(2):
                    ob0 = outsb.tile([P, 512], DT, tag="ob")
                    nc.scalar.activation(ob0[:], op01[ch][:], COPY, scale=recips[0][:, 0:1])
                    nc.sync.dma_start(
                        out=out_d[sbi * NIT * P:(sbi * NIT + 1) * P, ch * 512:(ch + 1) * 512],
                        in_=ob0[:])

                for gi in range(2, NIT * 2):
                    out_group(gi, recips)

    nc.compile()
    return nc


_NC_CACHE = None


def kernel(x, wq, wk, wv, wo):
    global _NC_CACHE
    if _NC_CACHE is None:
        _NC_CACHE = _build()
    nc = _NC_CACHE
    core_ids = list(range(N_CORES))
    wq16 = np.ascontiguousarray(wq, dtype=np.float16)
    wk16 = np.ascontiguousarray(wk, dtype=np.float16)
    wv16 = np.ascontiguousarray(wv, dtype=np.float16)
    wot16 = np.ascontiguousarray(wo.astype(np.float16).T)
    in_maps = []
    for b in range(N_CORES):
        in_maps.append({
            "xt": np.ascontiguousarray(x[b].astype(np.float16).T),
            "wq": wq16,
            "wk": wk16,
            "wv": wv16,
            "wot": wot16,
        })
    res = run_bass_kernel_spmd(nc, in_maps, core_ids)
    return np.stack([res.results[b]["out"] for b in range(N_CORES)], axis=0)
